# revision 5
# baseline (speedup 1.0000x reference)
"""Trainium2 Bass kernel for nn_DecoderModule_16853451669850 (8 NeuronCores).

Decoder block: x + MHA(x) -> LN -> +FFN -> LN.
Sharding: heads 2c,2c+1 on core c for attention (tensor-parallel over heads);
tokens [256c, 256c+256) on core c for pool+LN+FFN (sequence-parallel).
A single 8-way AllToAll (bf16) pivots between the two shardings.

Precision: float32r (e8m11) for the x/W_Q/W_K/Q/K/score path (the causal
softmax here is near-one-hot with huge logits, so bf16 flips argmaxes);
bf16 for V/P/pool/FFN operands; fp32 PSUM accumulation, softmax statistics,
layernorms and residuals.

Perf notes:
- All big weight tensors (W_Pool/W_1/W_2) stream through one 3-deep ring of
  [128,16,512] bf16 tiles, one dma_start per 2MB batch (the per-call SWDGE
  issue overhead of ~1us was the old bottleneck: 600 calls = 600us of Pool
  engine time).  Issue order matches consumption order; ring-slot deps give
  prefetch/pipelining automatically.
- The weight stream lives on the gpsimd queue; everything latency-critical
  (x streams, transposes, A2A staging, output store) is on sync/HWDGE so it
  can never queue behind a ring-stalled weight DMA.  The collective is placed
  on the gpsimd queue after only ring-slot-free batches, so it cannot
  deadlock against stream consumption.
- FFN1 computes its output directly transposed (hT[f-part, tok]) so FFN2
  needs no h transpose; bias+ReLU ride the scalar engine (per-partition bias).
"""

import numpy as np
import ml_dtypes
import concourse.bacc as bacc
import concourse.mybir as mybir
import concourse.tile as tile
from concourse.alu_op_type import AluOpType

F32, F32R, BF16 = mybir.dt.float32, mybir.dt.float32r, mybir.dt.bfloat16
AF = mybir.ActivationFunctionType
H, D, E, N, F = 16, 128, 2048, 2048, 8192
NCORE = 8
TOKPC = N // NCORE         # 256 tokens per core
SCALE = 1.0 / np.sqrt(np.float32(D))
EPS = 1e-5
NEG = -1.0e30


def build_nc(dbg=(), collective=True):
    nc = bacc.Bacc("TRN2", target_bir_lowering=False, debug=False)
    nc._use_collective = collective
    dt_in = {}

    def param(name, shape, dtype):
        dt_in[name] = dtype
        return nc.declare_dram_parameter(name, list(shape), dtype, isOutput=False)

    xT = param("xT", [E, N], F32R)            # x transposed, f32r-rounded
    wq = param("wq", [E, 2 * D], F32R)        # this core's 2 heads
    wk = param("wk", [E, 2 * D], F32R)
    wv = param("wv", [E, 2 * D], F32R)
    wpool = param("wpool", [H * D, E], BF16)
    w1 = param("w1", [E, F], BF16)
    w2 = param("w2", [F, E], BF16)
    b1t = param("b1t", [128, F // 128], F32)  # b1 as per-partition columns
    b2bc = param("b2bc", [128, E], F32)
    beta_bc = param("beta_bc", [128, E], F32)
    gcol = param("gcol", [128, 1], F32)
    xblk = param("xblk", [TOKPC, E], F32)     # this core's token rows of x
    maskc = param("maskc", [128, 128], F32)   # 0 if m<=n else -1e30 (n=part)
    maskt = param("maskt", [128, 128], F32)   # 0 if m<=n else -1e30 (m=part)
    ones1 = param("ones1", [1, 128], F32)
    epscol = param("epscol", [128, 1], F32)
    ident = param("ident", [128, 128], F32)

    out_blk = nc.declare_dram_parameter("out_blk", [TOKPC, E], F32, isOutput=True)
    dbg_outs = {}
    def dbg_param(name, shape, dtype=F32):
        if name in dbg:
            dbg_outs[name] = nc.declare_dram_parameter(name, list(shape), dtype, isOutput=True)
        return dbg_outs.get(name)

    dbg_param("d_qt", [128, N])
    dbg_param("d_heads", [128, N], BF16)
    dbg_param("d_pool", [TOKPC, E])

    with tile.TileContext(nc) as tc:
        _emit(nc, tc, locals())
    nc.compile()
    return nc, dt_in


def _emit(nc, tc, t):
    xT, wq, wk, wv = t["xT"], t["wq"], t["wk"], t["wv"]
    wpool, w1, w2 = t["wpool"], t["w1"], t["w2"]
    b1t, b2bc, beta_bc, gcol = t["b1t"], t["b2bc"], t["beta_bc"], t["gcol"]
    xblk, maskc, maskt, ones1, ident = t["xblk"], t["maskc"], t["maskt"], t["ones1"], t["ident"]
    epscol = t["epscol"]
    out_blk, dbg_outs = t["out_blk"], t["dbg_outs"]

    # ---- small persistent consts (gpsimd queue, before the weight stream) ----
    cp = tc.alloc_tile_pool(name="consts", bufs=1)
    c_mask = cp.tile([128, 128], F32); nc.gpsimd.dma_start(c_mask[:], maskc[:])
    c_maskt = cp.tile([128, 128], F32); nc.gpsimd.dma_start(c_maskt[:], maskt[:])
    c_ones = cp.tile([1, 128], F32); nc.gpsimd.dma_start(c_ones[:], ones1[:])
    c_id = cp.tile([128, 128], F32); nc.gpsimd.dma_start(c_id[:], ident[:])
    c_gcol = cp.tile([128, 1], F32); nc.gpsimd.dma_start(c_gcol[:], gcol[:])
    c_eps = cp.tile([128, 1], F32); nc.gpsimd.dma_start(c_eps[:], epscol[:])

    # attention-phase persistent tensors (released after the A2A staging-out)
    p_att = tc.alloc_tile_pool(name="p_att", bufs=1)
    QT = [p_att.tile([128, N], F32R, tag=f"qt{h}", name=f"QT{h}") for h in range(2)]
    KT = [p_att.tile([128, N], F32R, tag=f"kt{h}", name=f"KTt{h}") for h in range(2)]
    VT = [p_att.tile([128, N], BF16, tag=f"vt{h}", name=f"VTt{h}") for h in range(2)]
    Vn = [p_att.tile([128, 16, 128], BF16, tag=f"v{h}", name=f"Vn{h}") for h in range(2)]
    HT = [p_att.tile([128, N], BF16, tag=f"ht{h}", name=f"HTt{h}") for h in range(2)]

    # ---- phase 1: QKV projections (streams xT in 256-col chunks, sync/HWDGE) ----
    with tc.tile_pool(name="qkvw", bufs=1) as wp, \
         tc.tile_pool(name="xts", bufs=2) as xp, \
         tc.tile_pool(name="qkv_ps", bufs=1, space="PSUM") as qps:
        w_sb = {}
        for nm, src in (("q", wq), ("k", wk), ("v", wv)):
            w_sb[nm] = wp.tile([128, 16, 2 * D], F32R, tag="w" + nm, name="wsb_" + nm)
            nc.sync.dma_start(w_sb[nm][:], src[:].rearrange("(et ep) d -> ep et d", ep=128))
        for nch in range(8):
            xt_sb = xp.tile([128, 16, 256], F32R, tag="xt")
            nc.sync.dma_start(
                xt_sb[:], xT[:, nch * 256:(nch + 1) * 256].rearrange("(et ep) n -> ep et n", ep=128))
            ps = {}
            for nm in ("q", "k", "v"):
                for h in range(2):
                    ps[nm, h] = qps.tile([128, 256], F32, tag=f"ps{nm}{h}", name=f"ps_{nm}{h}")
            for et in range(16):
                for nm in ("q", "k", "v"):
                    for h in range(2):
                        nc.tensor.matmul(
                            ps[nm, h][:], w_sb[nm][:, et, h * 128:(h + 1) * 128],
                            xt_sb[:, et, :], start=(et == 0), stop=(et == 15))
            for h in range(2):
                sl = slice(nch * 256, (nch + 1) * 256)
                nc.vector.tensor_copy(QT[h][:, sl], ps["q", h][:])
                nc.vector.tensor_copy(KT[h][:, sl], ps["k", h][:])
                nc.vector.tensor_copy(VT[h][:, sl], ps["v", h][:])

    for h in range(2):
        nc.sync.dma_start_transpose(Vn[h][:], VT[h][:])

    if "d_qt" in dbg_outs: nc.sync.dma_start(dbg_outs["d_qt"][:], QT[0][:])

    # ---- weight stream: one tag ring, issue order == consumption order ----
    # wpool x4, w1 x16, w2 x16 batches of [128,16,512] bf16 (2MB each).
    wstream = tc.alloc_tile_pool(name="wstream", bufs=3, side="right")

    def ws_issue(src_ap):
        wt = wstream.tile([128, 16, 512], BF16, tag="ws")
        nc.gpsimd.dma_start(wt[:], src_ap)
        return wt

    # first two wpool batches fill free ring slots (safe before the collective)
    wpool_tiles = [
        ws_issue(wpool[:, ech * 512:(ech + 1) * 512].rearrange("(k p) e -> p k e", p=128))
        for ech in range(2)]

    # ---- phase 2: attention per head (single score pass) ----
    # Scores are computed once, in natural layout [n-part, m-free]; softmax
    # stats are per-partition there (cheap).  The normalized bf16 P row-block
    # is DMA-transposed into PT[m-part, mt, n] and AV accumulates from PT.
    # Per-head A2A staging + collective are issued as soon as that head's HT
    # is ready, so collective h=0 overlaps head h=1 compute.
    p_pool = tc.alloc_tile_pool(name="p_pool", bufs=1, side="right")
    plhs = p_pool.tile([128, 16 * TOKPC], BF16, tag="plhs")
    xb = p_pool.tile([128, 2 * E], F32, tag="xb")
    z = p_pool.tile([128, 2 * E], F32, tag="z")
    dp = tc.alloc_tile_pool(name="dramp", bufs=1, space="DRAM")
    a2a_in = [dp.tile([NCORE * 128, TOKPC], BF16, tag=f"a2ain{h}") for h in range(2)]
    a2a_out = [dp.tile([NCORE * 128, TOKPC], BF16, tag=f"a2aout{h}") for h in range(2)]

    with tc.tile_pool(name="att_sb", bufs=1) as asb, \
         tc.tile_pool(name="att_p", bufs=2) as app, \
         tc.tile_pool(name="snat_ps", bufs=4, space="PSUM") as sps, \
         tc.tile_pool(name="av_ps", bufs=2, space="PSUM") as tps:
        for h in range(2):
            maxcol = asb.tile([128, 16], F32, tag=f"maxcol{h}")
            sumcol = asb.tile([128, 16], F32, tag=f"sumcol{h}")
            rsumcol = asb.tile([128, 16], F32, tag=f"rsumcol{h}")
            bias1 = asb.tile([128, 16], F32, tag=f"bias1{h}")
            for nch in range(4):
                nmch = nch + 1
                PTt = app.tile([128, 16, 512], BF16, tag="PT")
                for nbw in range(4):
                    nb = 4 * nch + nbw
                    chunks = []
                    for mch in range(nmch):
                        sn = sps.tile([128, 512], F32, tag="snat")
                        nc.tensor.matmul(sn[:], QT[h][:, nb * 128:(nb + 1) * 128],
                                         KT[h][:, mch * 512:(mch + 1) * 512],
                                         start=True, stop=True)
                        chunks.append(sn)
                    nc.vector.tensor_tensor(chunks[-1][:, nbw * 128:(nbw + 1) * 128],
                                            chunks[-1][:, nbw * 128:(nbw + 1) * 128],
                                            c_mask[:], op=AluOpType.add)
                    mx = app.tile([128, 4], F32, tag="mx")
                    sm = app.tile([128, 4], F32, tag="sm")
                    wlast = nbw * 128 + 128
                    for mch in range(nmch):
                        w = 512 if mch < nmch - 1 else wlast
                        nc.vector.reduce_max(mx[:, mch:mch + 1], chunks[mch][:, 0:w],
                                             axis=mybir.AxisListType.X)
                    nc.vector.reduce_max(maxcol[:, nb:nb + 1], mx[:, 0:nmch],
                                         axis=mybir.AxisListType.X)
                    nc.vector.tensor_scalar_mul(bias1[:, nb:nb + 1],
                                                maxcol[:, nb:nb + 1], -float(SCALE))
                    Pnb = app.tile([128, N], BF16, tag="Pnb")
                    for mch in range(nmch):
                        w = 512 if mch < nmch - 1 else wlast
                        nc.scalar.activation(Pnb[:, mch * 512:mch * 512 + w],
                                             chunks[mch][:, 0:w], AF.Exp,
                                             bias=bias1[:, nb:nb + 1],
                                             scale=float(SCALE),
                                             accum_out=sm[:, mch:mch + 1])
                    nc.vector.reduce_sum(sumcol[:, nb:nb + 1], sm[:, 0:nmch],
                                         axis=mybir.AxisListType.X)
                    nc.vector.reciprocal(rsumcol[:, nb:nb + 1], sumcol[:, nb:nb + 1])
                    nc.vector.tensor_scalar_mul(Pnb[:, 0:(nch * 512 + wlast)],
                                                Pnb[:, 0:(nch * 512 + wlast)],
                                                rsumcol[:, nb:nb + 1])
                    if wlast < 512:
                        nc.vector.memset(Pnb[:, nch * 512 + wlast:nmch * 512], 0.0)
                    nc.sync.dma_start_transpose(
                        PTt[:, 0:nmch * 4, nbw * 128:(nbw + 1) * 128],
                        Pnb[:, 0:nmch * 512])
                av = tps.tile([128, 512], F32, tag="av")
                ntile = 4 * nch + 4
                for mt in range(ntile):
                    nc.tensor.matmul(av[:], Vn[h][:, mt, :], PTt[:, mt, :],
                                     start=(mt == 0), stop=(mt == ntile - 1),
                                     skip_group_check=True)
                nc.vector.tensor_copy(HT[h][:, nch * 512:(nch + 1) * 512], av[:])
            # stage + A2A this head as soon as it is done
            nc.sync.dma_start(
                a2a_in[h][:].rearrange("(j d) t -> d j t", d=128),
                HT[h][:].rearrange("p (j t) -> p j t", j=8))
            if getattr(nc, "_use_collective", True):
                nc.gpsimd.collective_compute(
                    "AllToAll", AluOpType.bypass,
                    ins=[a2a_in[h].opt()], outs=[a2a_out[h].opt()],
                    replica_groups=[list(range(NCORE))])
            else:
                nc.gpsimd.dma_start(a2a_out[h][:], a2a_in[h][:])
            # head h of core j lands in plhs column-block k = 2*j + h
            nc.sync.dma_start(
                plhs[:].rearrange("p (j two t) -> two p j t", two=2)[h],
                a2a_out[h][:].rearrange("(j d) t -> d j t", d=128))
    if "d_heads" in dbg_outs: nc.sync.dma_start(dbg_outs["d_heads"][:], HT[0][:])

    nc.sync.dma_start(xb[:].rearrange("p (nb e) -> p nb e", nb=2),
                      xblk[:].rearrange("(nb p) e -> p nb e", p=128))
    dp.release()
    p_att.release()

    # rest of the weight stream (ring-slot waits are fine from here on)
    wpool_tiles += [
        ws_issue(wpool[:, ech * 512:(ech + 1) * 512].rearrange("(k p) e -> p k e", p=128))
        for ech in range(2, 4)]
    w1_tiles = [
        ws_issue(w1[:, fch * 512:(fch + 1) * 512].rearrange("(et p) f -> p et f", p=128))
        for fch in range(16)]
    w2_tiles = [
        ws_issue(w2[ftg * 2048:(ftg + 1) * 2048, ech * 512:(ech + 1) * 512]
                 .rearrange("(ft p) e -> p ft e", p=128))
        for ech in range(4) for ftg in range(4)]

    # late consts + main-phase tensors
    p_main = tc.alloc_tile_pool(name="p_main", bufs=1)
    c_b1t = p_main.tile([128, F // 128], F32, tag="b1t")
    nc.gpsimd.dma_start(c_b1t[:], b1t[:])
    c_b2 = p_main.tile([128, E], F32, tag="b2")
    nc.gpsimd.dma_start(c_b2[:], b2bc[:])
    c_beta = p_main.tile([128, E], F32, tag="beta")
    nc.gpsimd.dma_start(c_beta[:], beta_bc[:])
    y = p_main.tile([128, 2 * E], BF16, tag="y")
    yT = p_main.tile([128, 16, 2 * 128], BF16, tag="yT")
    hT = p_main.tile([128, F // 128, TOKPC], BF16, tag="hT")
    z2 = p_main.tile([128, 2 * E], F32, tag="z2")
    out_t = p_main.tile([128, 2 * E], F32, tag="out")

    # ---- phase 4: pool + residual + LN1 ----
    with tc.tile_pool(name="pool_ps", bufs=4, space="PSUM") as pps:
        for ech in range(4):
            wsb = wpool_tiles[ech]
            for nb in range(2):
                pp = pps.tile([128, 512], F32, tag="pool")
                for k in range(16):
                    nc.tensor.matmul(pp[:], plhs[:, k * TOKPC + nb * 128:
                                                 k * TOKPC + (nb + 1) * 128],
                                     wsb[:, k, :], start=(k == 0), stop=(k == 15))
                sl = slice(nb * E + ech * 512, nb * E + (ech + 1) * 512)
                nc.vector.tensor_tensor(z[:, sl], pp[:], xb[:, sl], op=AluOpType.add)
    if "d_pool" in dbg_outs:
        nc.sync.dma_start(dbg_outs["d_pool"][:].rearrange("(nb p) e -> p nb e", p=128),
                          z[:].rearrange("p (nb e) -> p nb e", nb=2))
    _layernorm(nc, tc, z, y, c_gcol, c_beta, c_eps)
    for nb in range(2):
        nc.sync.dma_start_transpose(yT[:, :, nb * 128:(nb + 1) * 128],
                                    y[:, nb * E:(nb + 1) * E])
    p_pool.release()

    # ---- phase 5: FFN1 (output directly transposed: hT[f-part, tok]) ----
    with tc.tile_pool(name="f1_ps", bufs=4, space="PSUM") as f1ps:
        for fch in range(16):
            wsb = w1_tiles[fch]
            for fbw in range(4):
                fb = fch * 4 + fbw
                ps1 = f1ps.tile([128, TOKPC], F32, tag="f1")
                for et in range(16):
                    nc.tensor.matmul(ps1[:], wsb[:, et, fbw * 128:(fbw + 1) * 128],
                                     yT[:, et, :], start=(et == 0), stop=(et == 15))
                nc.scalar.activation(hT[:, fb, :], ps1[:], AF.Relu,
                                     bias=c_b1t[:, fb:fb + 1], scale=1.0)

    # ---- phase 6: FFN2 + residual + LN2 ----
    with tc.tile_pool(name="f2_ps", bufs=2, space="PSUM") as f2ps:
        for ech in range(4):
            ps2 = [f2ps.tile([128, 512], F32, tag=f"f2{_}", name=f"f2ps{_}")
                   for _ in range(2)]
            for ftg in range(4):
                wsb = w2_tiles[ech * 4 + ftg]
                for ftw in range(16):
                    fb = ftg * 16 + ftw
                    for nb in range(2):
                        nc.tensor.matmul(ps2[nb][:], hT[:, fb, nb * 128:(nb + 1) * 128],
                                         wsb[:, ftw, :], start=(fb == 0), stop=(fb == 63))
            for nb in range(2):
                sl = slice(nb * E + ech * 512, nb * E + (ech + 1) * 512)
                nc.vector.tensor_tensor(z2[:, sl], ps2[nb][:], y[:, sl],
                                        op=AluOpType.add)
                nc.vector.tensor_tensor(z2[:, sl], z2[:, sl],
                                        c_b2[:, ech * 512:(ech + 1) * 512],
                                        op=AluOpType.add)
    _layernorm(nc, tc, z2, out_t, c_gcol, c_beta, c_eps)
    nc.sync.dma_start(out_blk[:].rearrange("(nb p) e -> p nb e", p=128),
                      out_t[:].rearrange("p (nb e) -> p nb e", nb=2))
    p_main.release()
    wstream.release()
    cp.release()


def _layernorm(nc, tc, z, out, gcol, beta, epsc):
    with tc.tile_pool(name="lnp", bufs=2) as lp:
        for nb in range(2):
            stats = lp.tile([128, 4, 6], F32, tag="bnst")
            for ch in range(4):
                nc.vector.bn_stats(stats[:, ch, :],
                                   z[:, nb * E + ch * 512: nb * E + (ch + 1) * 512])
            mv = lp.tile([128, 2], F32, tag="bnag")
            nc.vector.bn_aggr(mv[:], stats[:])
            std = lp.tile([128, 1], F32, tag="std")
            nc.scalar.activation(std[:], mv[:, 1:2], AF.Sqrt, bias=epsc[:])
            rstd = lp.tile([128, 1], F32, tag="rstd")
            nc.vector.reciprocal(rstd[:], std[:])
            rg = lp.tile([128, 1], F32, tag="rg")
            nc.vector.tensor_tensor(rg[:], rstd[:], gcol[:], op=AluOpType.mult)
            sl = slice(nb * E, (nb + 1) * E)
            nc.vector.tensor_scalar(out[:, sl], z[:, sl], mv[:, 0:1], rg[:],
                                    AluOpType.subtract, AluOpType.mult)
            nc.vector.tensor_tensor(out[:, sl], out[:, sl], beta[:], op=AluOpType.add)


def round11(a):
    u = np.ascontiguousarray(a, dtype=np.float32).view(np.uint32).astype(np.uint64)
    return ((u + np.uint64(0x800)) & np.uint64(0xFFFFF000)).astype(np.uint32).view(np.float32)


def prep_inputs(inp):
    """Full reference inputs -> list of 8 per-core input dicts."""
    x = np.asarray(inp["token_embeddings"], np.float32)
    WQ = np.asarray(inp["W_Q"], np.float32); WK = np.asarray(inp["W_K"], np.float32)
    WV = np.asarray(inp["W_V"], np.float32); WP = np.asarray(inp["W_Pool"], np.float32)
    W1 = np.asarray(inp["W_1"], np.float32); b1 = np.asarray(inp["b_1"], np.float32)
    W2 = np.asarray(inp["W_2"], np.float32); b2 = np.asarray(inp["b_2"], np.float32)
    gamma = np.asarray(inp["gamma"], np.float32); beta = np.asarray(inp["beta"], np.float32)
    bf = ml_dtypes.bfloat16
    xT = round11(np.ascontiguousarray(x.T))
    shared = {
        "xT": xT,
        "wpool": WP.astype(bf),
        "w1": W1.astype(bf),
        "w2": W2.astype(bf),
        "b1t": np.ascontiguousarray(b1.reshape(F // 128, 128).T.astype(np.float32)),
        "b2bc": np.broadcast_to(b2.reshape(1, E), (128, E)).astype(np.float32).copy(),
        "beta_bc": np.broadcast_to(beta.reshape(1, E), (128, E)).astype(np.float32).copy(),
        "gcol": np.full((128, 1), float(gamma.reshape(-1)[0]), np.float32),
        "maskc": np.where(np.arange(128)[None, :] <= np.arange(128)[:, None], 0.0, NEG).astype(np.float32),
        "maskt": np.where(np.arange(128)[:, None] <= np.arange(128)[None, :], 0.0, NEG).astype(np.float32),
        "ones1": np.ones((1, 128), np.float32),
        "epscol": np.full((128, 1), EPS, np.float32),
        "ident": np.eye(128, dtype=np.float32),
    }
    maps = []
    for c in range(NCORE):
        m = dict(shared)
        m["wq"] = round11(np.concatenate([WQ[2 * c], WQ[2 * c + 1]], axis=1))
        m["wk"] = round11(np.concatenate([WK[2 * c], WK[2 * c + 1]], axis=1))
        m["wv"] = round11(np.concatenate([WV[2 * c], WV[2 * c + 1]], axis=1))
        m["xblk"] = np.ascontiguousarray(x[c * TOKPC:(c + 1) * TOKPC])
        maps.append(m)
    return maps


def assemble(results):
    return np.concatenate([r["out_blk"] for r in results], axis=0)


# ----------------------------------------------------------------------------
# PJRT execution (axon): jit once, reuse.
# ----------------------------------------------------------------------------
import jax
from concourse.bass2jax import _bass_exec_p, install_neuronx_cc_hook, partition_id_tensor
from jax.sharding import Mesh, PartitionSpec
from jax.experimental.shard_map import shard_map


class _Runner:
    def __init__(self, nc, n_cores):
        install_neuronx_cc_hook()
        self.nc = nc
        self.n_cores = n_cores
        in_names, out_names, out_avals, zero_outs = [], [], [], []
        for alloc in nc.m.functions[0].allocations:
            if not isinstance(alloc, mybir.MemoryLocationSet):
                continue
            name = alloc.memorylocations[0].name
            if alloc.kind == "ExternalInput":
                in_names.append(name)
            elif alloc.kind == "ExternalOutput":
                out_names.append(name)
                shape = tuple(alloc.tensor_shape)
                dtype = mybir.dt.np(alloc.dtype)
                out_avals.append(jax.core.ShapedArray(shape, dtype))
                zero_outs.append(np.zeros(shape, dtype))
        self.partition_name = nc.partition_id_tensor.name if nc.partition_id_tensor else None
        if self.partition_name in in_names:
            in_names.remove(self.partition_name)
        self.in_names = list(in_names)
        self.out_names = out_names
        self.out_avals = out_avals
        self.zero_outs = zero_outs
        self.n_params = len(in_names)
        all_in_names = in_names + out_names
        if self.partition_name is not None:
            all_in_names.append(self.partition_name)
        partition_name = self.partition_name

        def _body(*args):
            operands = list(args)
            if partition_name is not None:
                operands.append(partition_id_tensor())
            outs = _bass_exec_p.bind(
                *operands,
                out_avals=tuple(out_avals),
                in_names=tuple(all_in_names),
                out_names=tuple(out_names),
                lowering_input_output_aliases=(),
                sim_require_finite=True,
                sim_require_nnan=True,
                nc=nc,
            )
            return tuple(outs)

        devices = jax.devices()[:n_cores]
        self.mesh = Mesh(np.asarray(devices), ("core",))
        n_outs = len(out_avals)
        in_specs = (PartitionSpec("core"),) * (self.n_params + n_outs)
        out_specs = (PartitionSpec("core"),) * len(out_names)
        self.fn = jax.jit(
            shard_map(_body, mesh=self.mesh, in_specs=in_specs,
                      out_specs=out_specs, check_rep=False),
            keep_unused=True)

    def prep(self, in_maps):
        per_core = [[np.asarray(m[n]) for n in self.in_names] for m in in_maps]
        concat_in = [np.concatenate([per_core[c][i] for c in range(self.n_cores)], axis=0)
                     for i in range(self.n_params)]
        concat_zeros = [np.zeros((self.n_cores * z.shape[0], *z.shape[1:]), z.dtype)
                        for z in self.zero_outs]
        sh = jax.sharding.NamedSharding(self.mesh, PartitionSpec("core"))
        return [jax.device_put(a, sh) for a in concat_in + concat_zeros]

    def run(self, args):
        outs = self.fn(*args)
        jax.block_until_ready(outs)
        return outs

    def results(self, outs):
        return [
            {n: np.asarray(outs[i]).reshape(self.n_cores, *self.out_avals[i].shape)[c]
             for i, n in enumerate(self.out_names)}
            for c in range(self.n_cores)
        ]


_CACHE = {}


def _get_runner():
    if "r" not in _CACHE:
        nc, _ = build_nc()
        _CACHE["r"] = _Runner(nc, NCORE)
    return _CACHE["r"]


def kernel(**inputs):
    r = _get_runner()
    maps = prep_inputs(inputs)
    args = r.prep(maps)
    outs = r.run(args)
    return assemble(r.results(outs)).astype(np.float32)
